# revision 19
# baseline (speedup 1.0000x reference)
"""DepthLoss kernel for 8 Trainium2 NeuronCores.

reference:
    rows/cols/d = rdepth[...,0/1/2]; mask = d>0
    vals = output[b, 0, rows, cols]
    loss = sum(mask * |vals - d|) / max(count(mask), 1)   (0 if count==0)

Strategy: data-parallel over batch (4 planes/core). Per core:
  - compute pixel index pix = r*W + c on DVE; split into a 64-element
    row id (int16) and a within-row offset cmod
  - dma_gather (SWDGE bulk gather) fetches each sample's 256B image row
  - one-hot select on DVE picks the target element out of each row
  - masked |v - d| partial sums + counts per partition -> [128, 2]
Host combines the 8 cores' partials and does the final divide.

Index bookkeeping: dma_gather consumes index i from partition i%16,
column i//16 of its idx tile and writes the row to G[i%128, i//128, :].
With row-ids for sample s = 1024*q + 128*(u%8) + u//8 stored at idx
tile [q, u], the gather output G[p, jj] holds sample 128*pi(p) + jj
where pi(p) = 8*(p%16) + p//16.  Loading the per-batch rdepth with a
permuted-partition AP (partition p <- contiguous samples starting at
128*pi(p)) makes d/cmod line up with G with no cross-partition moves.
"""

import numpy as np

import concourse.bacc as bacc
import concourse.mybir as mybir
import concourse.tile as tile
from concourse import library_config
from concourse.bass_utils import run_bass_kernel_spmd

B, N, H, W = 32, 16384, 768, 1024
NCORES = 8
BPC = B // NCORES          # batches (planes) per core = 4
P = 128
PLANE = H * W              # 786432
E = 64                     # gathered row length (f32) = 256 B
RT = PLANE // E            # rows per plane table = 12288
U = N // 16                # idx columns = 1024
JJ = N // P                # samples per partition per batch = 128
F32 = mybir.dt.float32
I16 = mybir.dt.int16
I32 = mybir.dt.int32
Alu = mybir.AluOpType
AX = mybir.AxisListType


def build(n_iters=1):
    nc = bacc.Bacc(
        "TRN2", target_bir_lowering=False, debug=False,
        num_swdge_queues=4,
    )

    img = nc.dram_tensor("img", [BPC * RT, E], F32, kind="ExternalInput")
    rdp = nc.dram_tensor("rdp", [BPC * N, 3], F32, kind="ExternalInput")
    out = nc.dram_tensor("out", [P, 2], F32, kind="ExternalOutput")

    with tile.TileContext(nc) as tc:
        with (
            tc.tile_pool(name="const", bufs=1) as cst,
            tc.tile_pool(name="acc", bufs=1) as acc,
            tc.tile_pool(name="big", bufs=2) as big,
            tc.tile_pool(name="sm", bufs=2) as sm,
        ):
            nc.gpsimd.load_library(library_config.mlp)
            # iota64f[p, m] = m for m in 0..63
            io_i = cst.tile([P, E], I32, tag="io_i")
            nc.gpsimd.iota(io_i[:], pattern=[[1, E]], channel_multiplier=0)
            iota64 = cst.tile([P, E], F32, tag="iota64")
            nc.vector.tensor_copy(out=iota64[:], in_=io_i[:])

            for _ in range(n_iters):
                # --- idx prep: rt16[32b+q, 3u+c] = rdepth[b, 1024q+u, c]
                # (batch b on partitions [32b, 32b+16) so later per-batch
                #  DVE reads start at a legal quadrant base)
                rt16 = acc.tile([P, 3 * U], F32, tag="rt16")
                nc.gpsimd.memset(rt16[:], 0)
                for b in range(BPC):
                    nc.sync.dma_start(
                        out=rt16[32 * b : 32 * b + 16, :],
                        in_=rdp[b * N : (b + 1) * N, :].rearrange(
                            "(q u) c -> q (u c)", q=16
                        ),
                    )
                rv16 = rt16[:].rearrange("p (u c) -> p u c", c=3)
                pix = acc.tile([P, U], F32, tag="pix")
                nc.vector.tensor_scalar(
                    out=pix[:], in0=rv16[:, :, 0], scalar1=float(W),
                    scalar2=None, op0=Alu.mult,
                )
                nc.vector.tensor_tensor(
                    out=pix[:], in0=pix[:], in1=rv16[:, :, 1], op=Alu.add
                )
                pixi = acc.tile([P, U], I32, tag="pixi")
                nc.vector.tensor_copy(out=pixi[:], in_=pix[:])
                rowi = acc.tile([P, U], I32, tag="rowi")
                nc.vector.tensor_scalar(
                    out=rowi[:], in0=pixi[:], scalar1=6, scalar2=None,
                    op0=Alu.arith_shift_right,
                )

                lc = acc.tile([P, BPC], F32, tag="lc")
                cc2 = acc.tile([P, BPC], F32, tag="cc2")
                for b in range(BPC):
                    qs = slice(32 * b, 32 * b + 16)
                    # int16 row ids: idx16[q, u] = rowi[32b+q, 128*(u%8)+u//8],
                    # replicated across all 8 gpsimd-core stripes
                    idx16 = sm.tile([P, U], I16, tag="idx16")
                    nc.vector.tensor_copy(
                        out=idx16[0:16, :].rearrange("q (a e) -> q a e", e=8),
                        in_=rowi[qs, :]
                        .rearrange("q (e a) -> q e a", e=8)
                        .transpose([0, 2, 1]),
                    )
                    nc.sync.dma_start(out=idx16[16:32, :], in_=idx16[0:16, :])
                    nc.sync.dma_start(out=idx16[32:64, :], in_=idx16[0:32, :])
                    nc.sync.dma_start(out=idx16[64:128, :], in_=idx16[0:64, :])

                    # gather: G[p, jj, :] = img row of sample 128*pi(p)+jj
                    # (chunked so each SWDGE op fits the descriptor ring)
                    g = big.tile([P, JJ * E], F32, tag="G")
                    g3 = g[:].rearrange("p (j e) -> p j e", e=E)
                    NCH = 4
                    CI = N // NCH          # idxs per chunk = 4096
                    CJ = JJ // NCH         # dst cols per chunk = 32
                    CU = U // NCH          # idx tile cols per chunk
                    for k in range(NCH):
                        nc.gpsimd.dma_gather(
                            g3[:, k * CJ : (k + 1) * CJ, :],
                            img[b * RT : (b + 1) * RT, :],
                            idx16[:, k * CU : (k + 1) * CU],
                            CI,
                            CI,
                            E,
                            single_packet=False,
                            queue_num=k % 4,
                        )

                    # per-batch rdepth in gather layout:
                    # rtb[p, 3t+c] = rdepth[b, 128*pi(p)+t, c]
                    rtb = sm.tile([P, 3 * JJ], F32, tag="rtb")
                    src = bacc.bass.AP(
                        rdp,
                        b * N * 3,
                        [[3 * JJ, 8], [8 * 3 * JJ, 16], [1, 3 * JJ]],
                    )
                    nc.sync.dma_start(out=rtb[:], in_=src)
                    rvb = rtb[:].rearrange("p (t c) -> p t c", c=3)
                    dsel = rvb[:, :, 2]

                    pixb = sm.tile([P, JJ], F32, tag="pixb")
                    nc.vector.tensor_scalar(
                        out=pixb[:], in0=rvb[:, :, 0], scalar1=float(W),
                        scalar2=None, op0=Alu.mult,
                    )
                    nc.vector.tensor_tensor(
                        out=pixb[:], in0=pixb[:], in1=rvb[:, :, 1], op=Alu.add
                    )
                    pixbi = sm.tile([P, JJ], I32, tag="pixbi")
                    nc.vector.tensor_copy(out=pixbi[:], in_=pixb[:])
                    cmodi = sm.tile([P, JJ], I32, tag="cmodi")
                    nc.vector.tensor_scalar(
                        out=cmodi[:], in0=pixbi[:], scalar1=E - 1,
                        scalar2=None, op0=Alu.bitwise_and,
                    )
                    # csel = cmod + 1, in f32
                    csel = sm.tile([P, JJ], F32, tag="csel")
                    nc.vector.tensor_scalar(
                        out=csel[:], in0=cmodi[:], scalar1=1, scalar2=None,
                        op0=Alu.add,
                    )
                    # fold mask: csel = csel*(d>0) - 1
                    msel = sm.tile([P, JJ], F32, tag="msel")
                    nc.vector.tensor_scalar(
                        out=msel[:], in0=dsel, scalar1=0.0, scalar2=None,
                        op0=Alu.is_gt,
                    )
                    nc.vector.tensor_tensor(
                        out=csel[:], in0=csel[:], in1=msel[:], op=Alu.mult
                    )
                    nc.vector.tensor_scalar(
                        out=csel[:], in0=csel[:], scalar1=-1.0, scalar2=None,
                        op0=Alu.add,
                    )

                    # one-hot select: W = (iota64 == csel); v = sum(G*W)
                    w = big.tile([P, JJ * E], F32, tag="W")
                    w3 = w[:].rearrange("p (j e) -> p j e", e=E)
                    nc.vector.tensor_tensor(
                        out=w3,
                        in0=iota64[:].unsqueeze(1).to_broadcast([P, JJ, E]),
                        in1=csel[:].unsqueeze(2).to_broadcast([P, JJ, E]),
                        op=Alu.is_equal,
                    )
                    nc.vector.tensor_tensor(
                        out=w[:], in0=g[:], in1=w[:], op=Alu.mult
                    )
                    vsel = sm.tile([P, JJ], F32, tag="vsel")
                    nc.vector.tensor_reduce(
                        out=vsel[:], in_=w3, axis=AX.X, op=Alu.add
                    )

                    # masked |v - d| and count
                    diff = sm.tile([P, JJ], F32, tag="diff")
                    nc.vector.tensor_tensor(
                        out=diff[:], in0=vsel[:], in1=dsel, op=Alu.subtract
                    )
                    nc.vector.tensor_tensor(
                        out=diff[:], in0=diff[:], in1=msel[:], op=Alu.mult
                    )
                    nc.vector.tensor_reduce(
                        out=lc[:, b : b + 1], in_=diff[:], axis=AX.X,
                        op=Alu.add, apply_absolute_value=True,
                    )
                    nc.vector.tensor_reduce(
                        out=cc2[:, b : b + 1], in_=msel[:], axis=AX.X,
                        op=Alu.add,
                    )

                losscnt = acc.tile([P, 2], F32, tag="losscnt")
                nc.vector.tensor_reduce(
                    out=losscnt[:, 0:1], in_=lc[:], axis=AX.X, op=Alu.add
                )
                nc.vector.tensor_reduce(
                    out=losscnt[:, 1:2], in_=cc2[:], axis=AX.X, op=Alu.add
                )
                nc.sync.dma_start(out=out[:, :], in_=losscnt[:])

    nc.compile()
    return nc


_NC = None


def _get_nc():
    global _NC
    if _NC is None:
        _NC = build()
    return _NC


def make_in_maps(output, rdepth):
    in_maps = []
    for c in range(NCORES):
        sl = slice(c * BPC, (c + 1) * BPC)
        img_c = np.ascontiguousarray(
            output[sl, 0], dtype=np.float32
        ).reshape(BPC * RT, E)
        rdp_c = np.ascontiguousarray(
            rdepth[sl], dtype=np.float32
        ).reshape(BPC * N, 3)
        in_maps.append({"img": img_c, "rdp": rdp_c})
    return in_maps


def combine(results):
    partials = np.stack([r["out"] for r in results])  # [8, 128, 2]
    loss = partials[..., 0].astype(np.float64).sum()
    cnt = partials[..., 1].astype(np.float64).sum()
    val = loss / max(cnt, 1.0) if cnt > 0 else 0.0
    return np.asarray(val, dtype=np.float32)


def run(output, rdepth, **kw):
    res = run_bass_kernel_spmd(
        _get_nc(), make_in_maps(output, rdepth), list(range(NCORES)), **kw
    )
    return combine(res.results), res


def kernel(output, rdepth):
    return run(output, rdepth)[0]


# revision 32
# speedup vs baseline: 4.7869x; 4.7869x over previous
"""DepthLoss kernel for 8 Trainium2 NeuronCores.

reference:
    rows/cols/d = rdepth[...,0/1/2]; mask = d>0
    vals = output[b, 0, rows, cols]
    loss = sum(mask * |vals - d|) / max(count(mask), 1)   (0 if count==0)

Strategy: data-parallel over batch (4 planes/core). Per core:
  - compute pixel index pix = r*W + c on DVE; split into a 64-element
    row id (int16) and a within-row offset cmod
  - dma_gather (SWDGE bulk gather) fetches each sample's 256B image row
  - one-hot select on DVE picks the target element out of each row
  - masked |v - d| partial sums + counts per partition -> [128, 2]
Host combines the 8 cores' partials and does the final divide.

Index bookkeeping: dma_gather consumes index i from partition i%16,
column i//16 of its idx tile and writes the row to G[i%128, i//128, :].
With row-ids for sample s = 1024*q + 128*(u%8) + u//8 stored at idx
tile [q, u], the gather output G[p, jj] holds sample 128*pi(p) + jj
where pi(p) = 8*(p%16) + p//16.  Loading the per-batch rdepth with a
permuted-partition AP (partition p <- contiguous samples starting at
128*pi(p)) makes d/cmod line up with G with no cross-partition moves.
"""

import numpy as np

import concourse.bacc as bacc
import concourse.mybir as mybir
import concourse.tile as tile
from concourse import library_config
from concourse.bass_utils import run_bass_kernel_spmd

B, N, H, W = 32, 16384, 768, 1024
NCORES = 8
BPC = B // NCORES          # batches (planes) per core = 4
P = 128
PLANE = H * W              # 786432
E = 64                     # gathered row length (f32) = 256 B
RT = PLANE // E            # rows per plane table = 12288
U = N // 16                # idx columns = 1024
JJ = N // P                # samples per partition per batch = 128
F32 = mybir.dt.float32
I16 = mybir.dt.int16
I32 = mybir.dt.int32
BF16 = mybir.dt.bfloat16
Alu = mybir.AluOpType
AX = mybir.AxisListType


def build(n_iters=1, init_unused=True):
    nc = bacc.Bacc(
        "TRN2", target_bir_lowering=False, debug=False,
        num_swdge_queues=4,
    )

    img = nc.dram_tensor("img", [BPC * RT, E], F32, kind="ExternalInput")
    rdp = nc.dram_tensor("rdp", [BPC * N, 3], F32, kind="ExternalInput")
    out = nc.dram_tensor("out", [P, 2], F32, kind="ExternalOutput")

    with tile.TileContext(nc) as tc:
        with (
            tc.tile_pool(name="const", bufs=1) as cst,
            tc.tile_pool(name="acc", bufs=1) as acc,
            tc.tile_pool(name="big", bufs=2) as big,
            tc.tile_pool(name="wp", bufs=2) as wp,
            tc.tile_pool(name="sm", bufs=2) as sm,
        ):
            nc.gpsimd.load_library(library_config.mlp)
            # iota64f[p, m] = m for m in 0..63
            io_i = cst.tile([P, E], I32, tag="io_i")
            nc.gpsimd.iota(io_i[:], pattern=[[1, E]], channel_multiplier=0)
            iota64 = cst.tile([P, E], F32, tag="iota64")
            nc.vector.tensor_copy(out=iota64[:], in_=io_i[:])

            for _ in range(n_iters):
                # --- idx prep: rt16[32b+q, 3u+c] = rdepth[b, 1024q+u, c]
                # (batch b on partitions [32b, 32b+16) so later per-batch
                #  DVE reads start at a legal quadrant base)
                rt16 = acc.tile([P, 3 * U], F32, tag="rt16")
                if init_unused:
                    # only CoreSim needs this: the unwritten partition bands
                    # feed ops whose outputs are never consumed
                    nc.vector.memset(rt16[:], 0)
                for b in range(BPC):
                    nc.sync.dma_start(
                        out=rt16[32 * b : 32 * b + 16, :],
                        in_=rdp[b * N : (b + 1) * N, :].rearrange(
                            "(q u) c -> q (u c)", q=16
                        ),
                    )
                rv16 = rt16[:].rearrange("p (u c) -> p u c", c=3)
                pix = acc.tile([P, U], F32, tag="pix")
                nc.vector.tensor_scalar(
                    out=pix[:], in0=rv16[:, :, 0], scalar1=float(W),
                    scalar2=None, op0=Alu.mult,
                )
                nc.vector.tensor_tensor(
                    out=pix[:], in0=pix[:], in1=rv16[:, :, 1], op=Alu.add
                )
                pixi = acc.tile([P, U], I32, tag="pixi")
                nc.vector.tensor_copy(out=pixi[:], in_=pix[:])
                rowi = acc.tile([P, U], I32, tag="rowi")
                nc.vector.tensor_scalar(
                    out=rowi[:], in0=pixi[:], scalar1=6, scalar2=None,
                    op0=Alu.arith_shift_right,
                )

                lc = acc.tile([P, BPC], F32, tag="lc")
                cc2 = acc.tile([P, BPC], F32, tag="cc2")
                gs = []
                for b in range(BPC):
                    qs = slice(32 * b, 32 * b + 16)
                    # int16 row ids: idx16[q, u] = rowi[32b+q, 128*(u%8)+u//8],
                    # replicated across all 8 gpsimd-core stripes
                    idx16 = sm.tile([P, U], I16, tag=f"idx16_{b}")
                    nc.vector.tensor_copy(
                        out=idx16[0:16, :]
                        .rearrange("q (a e) -> q a e", e=8)
                        .transpose([0, 2, 1]),
                        in_=rowi[qs, :].rearrange("q (e a) -> q e a", e=8),
                    )
                    nc.sync.dma_start(out=idx16[16:32, :], in_=idx16[0:16, :])
                    nc.sync.dma_start(out=idx16[32:64, :], in_=idx16[0:32, :])
                    nc.sync.dma_start(out=idx16[64:128, :], in_=idx16[0:64, :])

                    # gather: G[p, jj, :] = img row of sample 128*pi(p)+jj
                    # (chunked so each SWDGE op fits the descriptor ring)
                    g = big.tile([P, JJ * E], F32, tag="G")
                    g3 = g[:].rearrange("p (j e) -> p j e", e=E)
                    gs.append((g, g3))
                    NCH = 16
                    CI = N // NCH          # idxs per chunk = 4096
                    CJ = JJ // NCH         # dst cols per chunk = 32
                    CU = U // NCH          # idx tile cols per chunk
                    for k in range(NCH):
                        nc.gpsimd.dma_gather(
                            g3[:, k * CJ : (k + 1) * CJ, :],
                            img[b * RT : (b + 1) * RT, :],
                            idx16[:, k * CU : (k + 1) * CU],
                            CI,
                            CI,
                            E,
                            single_packet=False,
                            queue_num=k % 4,
                        )

                for b in range(BPC):
                    g, g3 = gs[b]
                    # per-batch rdepth in gather layout:
                    # rtb[p, 3t+c] = rdepth[b, 128*pi(p)+t, c]
                    rtb = sm.tile([P, 3 * JJ], F32, tag="rtb")
                    src = bacc.bass.AP(
                        rdp,
                        b * N * 3,
                        [[3 * JJ, 8], [8 * 3 * JJ, 16], [1, 3 * JJ]],
                    )
                    nc.sync.dma_start(out=rtb[:], in_=src)
                    rvb = rtb[:].rearrange("p (t c) -> p t c", c=3)
                    dsel = rvb[:, :, 2]

                    pixb = sm.tile([P, JJ], F32, tag="pixb")
                    nc.vector.tensor_scalar(
                        out=pixb[:], in0=rvb[:, :, 0], scalar1=float(W),
                        scalar2=None, op0=Alu.mult,
                    )
                    nc.vector.tensor_tensor(
                        out=pixb[:], in0=pixb[:], in1=rvb[:, :, 1], op=Alu.add
                    )
                    pixbi = sm.tile([P, JJ], I32, tag="pixbi")
                    nc.vector.tensor_copy(out=pixbi[:], in_=pixb[:])
                    cmodi = sm.tile([P, JJ], I32, tag="cmodi")
                    nc.vector.tensor_scalar(
                        out=cmodi[:], in0=pixbi[:], scalar1=E - 1,
                        scalar2=None, op0=Alu.bitwise_and,
                    )
                    # csel = cmod + 1, in f32
                    csel = sm.tile([P, JJ], F32, tag="csel")
                    nc.vector.tensor_scalar(
                        out=csel[:], in0=cmodi[:], scalar1=1, scalar2=None,
                        op0=Alu.add,
                    )
                    # fold mask: csel = csel*(d>0) - 1
                    msel = sm.tile([P, JJ], F32, tag="msel")
                    nc.vector.tensor_scalar(
                        out=msel[:], in0=dsel, scalar1=0.0, scalar2=None,
                        op0=Alu.is_gt,
                    )
                    nc.vector.tensor_tensor(
                        out=csel[:], in0=csel[:], in1=msel[:], op=Alu.mult
                    )
                    nc.vector.tensor_scalar(
                        out=csel[:], in0=csel[:], scalar1=-1.0, scalar2=None,
                        op0=Alu.add,
                    )

                    # one-hot select: W = (iota64 == csel); v = sum(G*W)
                    # split into jj-halves so each half's mul starts as soon
                    # as its 8 gather chunks land (subtile deps)
                    w = wp.tile([P, JJ * E], F32, tag="W")
                    w3 = w[:].rearrange("p (j e) -> p j e", e=E)
                    vsel = sm.tile([P, JJ], F32, tag="vsel")
                    HJ = JJ // 2
                    for h in range(2):
                        js = slice(h * HJ, (h + 1) * HJ)
                        nc.vector.tensor_tensor(
                            out=w3[:, js, :],
                            in0=iota64[:].unsqueeze(1).to_broadcast(
                                [P, HJ, E]
                            ),
                            in1=csel[:, js].unsqueeze(2).to_broadcast(
                                [P, HJ, E]
                            ),
                            op=Alu.is_equal,
                        )
                        nc.vector.tensor_tensor(
                            out=w3[:, js, :], in0=g3[:, js, :],
                            in1=w3[:, js, :], op=Alu.mult,
                        )
                        nc.vector.tensor_reduce(
                            out=vsel[:, js], in_=w3[:, js, :], axis=AX.X,
                            op=Alu.add,
                        )

                    # masked |v - d| and count
                    diff = sm.tile([P, JJ], F32, tag="diff")
                    nc.vector.tensor_tensor(
                        out=diff[:], in0=vsel[:], in1=dsel, op=Alu.subtract
                    )
                    nc.vector.tensor_tensor(
                        out=diff[:], in0=diff[:], in1=msel[:], op=Alu.mult
                    )
                    nc.vector.tensor_reduce(
                        out=lc[:, b : b + 1], in_=diff[:], axis=AX.X,
                        op=Alu.add, apply_absolute_value=True,
                    )
                    nc.vector.tensor_reduce(
                        out=cc2[:, b : b + 1], in_=msel[:], axis=AX.X,
                        op=Alu.add,
                    )

                losscnt = acc.tile([P, 2], F32, tag="losscnt")
                nc.vector.tensor_reduce(
                    out=losscnt[:, 0:1], in_=lc[:], axis=AX.X, op=Alu.add
                )
                nc.vector.tensor_reduce(
                    out=losscnt[:, 1:2], in_=cc2[:], axis=AX.X, op=Alu.add
                )
                nc.sync.dma_start(out=out[:, :], in_=losscnt[:])

    nc.compile()
    return nc


_NC = None


def _get_nc():
    global _NC
    if _NC is None:
        _NC = build(init_unused=False)
    return _NC


def make_in_maps(output, rdepth):
    in_maps = []
    for c in range(NCORES):
        sl = slice(c * BPC, (c + 1) * BPC)
        img_c = np.ascontiguousarray(
            output[sl, 0], dtype=np.float32
        ).reshape(BPC * RT, E)
        rdp_c = np.ascontiguousarray(
            rdepth[sl], dtype=np.float32
        ).reshape(BPC * N, 3)
        in_maps.append({"img": img_c, "rdp": rdp_c})
    return in_maps


def combine(results):
    partials = np.stack([r["out"] for r in results])  # [8, 128, 2]
    loss = partials[..., 0].astype(np.float64).sum()
    cnt = partials[..., 1].astype(np.float64).sum()
    val = loss / max(cnt, 1.0) if cnt > 0 else 0.0
    return np.asarray(val, dtype=np.float32)


def run(output, rdepth, **kw):
    res = run_bass_kernel_spmd(
        _get_nc(), make_in_maps(output, rdepth), list(range(NCORES)), **kw
    )
    return combine(res.results), res


def kernel(output, rdepth):
    return run(output, rdepth)[0]
